# revision 22
# baseline (speedup 1.0000x reference)
"""Cross-attention kernel for Trainium2 (8 NeuronCores, SPMD).

Problem: out = x_a + gamma * attn_out where
  q = Wq @ xa + bq   [B, N, CK]     (1x1 conv == per-pixel linear)
  k = Wk @ xb + bk   [B, CK, N]
  v = Wv @ xb + bv   [B, N, C]
  attn_out = softmax(q @ k, axis=-1) @ v   (transposed back to [B, C, H, W])
with B=4, C=256, CK=32, N=64*64=4096.

Sharding: 8 cores = (batch b, n-half) pairs. Each core computes q for its
2048 rows, full k/v for its batch (replicated work within a batch pair),
and its 2048xN attention rows locally. No cross-core communication.

On-core dataflow (all matmuls in float32r, 1 PE cycle/row):
  All weights/biases arrive as ONE host-packed, host-zero-padded tensor
  (wpack [C, 514] = [WqT|0pad | WkT|0pad | WvT | bq | bk]) so SBUF setup
  needs two DMAs and zero memsets; per-DMA queue overhead (~0.6us each,
  serialized) is what gates the first matmul.
  qT [128, n] = WqT_pad.T @ xa    kmat [128, m] = WkT_pad.T @ xb
  (padding the weights' free dim to K=128 keeps the fast K=128 PE path
  and writes all 128 partitions of qtp/kmat directly)
  ST tile [m=128, n=512] pairs = kmat_block.T @ qT_pad
  expST = Exp(ST) on ScalarE (softmax without max-subtraction: logits are
  bounded ~|47| here, exp stays finite in fp32)
  out_aug [n, 258] += expST.T @ v_aug  where v_aug = [v | ones | pad]
  => column 256 accumulates the softmax denominator for free.
  finalize: scale rows by gamma/rowsum (gpsimd), PE-transpose to [c, n],
  add x_a (f32 view of the xa tile already in SBUF), DMA out in [C, N]
  layout.
DMA: xa on the SP HWDGE queue, xb on the Activation HWDGE queue, in
[128, 1024] chunks in consumption order.
"""
import numpy as np

import concourse.bass as bass
import concourse.mybir as mybir
import concourse.tile as tile
from concourse import bacc, bass_utils
from concourse.masks import make_identity

F32 = mybir.dt.float32
F32R = mybir.dt.float32r
EXP = mybir.ActivationFunctionType.Exp
CPY = mybir.ActivationFunctionType.Identity

B, C, H, W = 4, 256, 64, 64
N = H * W            # 4096 keys per batch
CK = 32              # q/k projection dim
NH = N // 2          # 2048 query rows per core
N_CORES = 8
NCH = NH // 512      # 4 n-chunks of 512 per core
MT = N // 128        # 32 m-tiles of 128
WP = 514             # packed weights: 128 wq | 128 wk | 256 wv | bq | bk


def _build():
    nc = bacc.Bacc("TRN2", target_bir_lowering=False, debug=False,
                   enable_asserts=False)
    xa = nc.dram_tensor("xa", [C, NH], F32R, kind="ExternalInput").ap()
    xb = nc.dram_tensor("xb", [C, N], F32R, kind="ExternalInput").ap()
    wpack = nc.dram_tensor("wpack", [C, WP], F32R, kind="ExternalInput").ap()
    bvg = nc.dram_tensor("bvg", [1, 259], F32, kind="ExternalInput").ap()
    out = nc.dram_tensor("out", [C, NH], F32, kind="ExternalOutput").ap()

    with tile.TileContext(nc) as tc:
        with tc.tile_pool(name="const", bufs=1) as const, \
             tc.tile_pool(name="work", bufs=3) as work, \
             tc.tile_pool(name="outp", bufs=2) as outp, \
             tc.tile_pool(name="small", bufs=4) as small, \
             tc.tile_pool(name="stp", bufs=4, space="PSUM") as stp, \
             tc.tile_pool(name="opp", bufs=1, space="PSUM") as opp:

            # ---- constants / persistent tiles -------------------------
            xa_sb = [const.tile([128, NH], F32R, tag=f"xa{h}", name=f"xa_sb{h}") for h in range(2)]
            xb_sb = [const.tile([128, N], F32R, tag=f"xb{h}", name=f"xb_sb{h}") for h in range(2)]
            wp_sb = [const.tile([128, WP], F32R, tag=f"wp{h}", name=f"wp_sb{h}") for h in range(2)]
            CS = [slice(0, 128), slice(128, 256)]
            kmat = const.tile([128, N], F32R, tag="kmat")
            qtp = const.tile([128, NH], F32R, tag="qtp")
            v_aug = const.tile([128, MT, 258], F32R, tag="vaug")
            bvg_sb = const.tile([128, 259], F32, tag="bvg")
            ident = const.tile([128, 128], F32, tag="ident")

            wqt_sb = [wp_sb[h][:, 0:128] for h in range(2)]
            wkt_sb = [wp_sb[h][:, 128:256] for h in range(2)]
            wvt_sb = [wp_sb[h][:, 256:512] for h in range(2)]
            bq_sb = wp_sb[0].bitcast(F32)[:, 512:513]
            bk_sb = wp_sb[0].bitcast(F32)[:, 513:514]
            bva_sb = bvg_sb[:, 0:258]
            gam_sb = bvg_sb[:, 258:259]

            make_identity(nc, ident)

            # Two HWDGE queues, DMAs in consumption order with small leading
            # chunks: first ST pair needs wp + xa[:512] + xb[:512] + bvg only.
            def ldx(eng, dst, src, h, lo, hi):
                eng.dma_start(out=dst[h][:, lo:hi], in_=src[CS[h], lo:hi])
            for h in range(2):
                nc.sync.dma_start(out=wp_sb[h], in_=wpack[CS[h], :])
            for h in range(2):
                ldx(nc.sync, xa_sb, xa, h, 0, 512)        # q-proj ch0
            for h in range(2):
                ldx(nc.sync, xb_sb, xb, h, 512, 1024)     # kt(1)/v(4..7)
            for h in range(2):
                ldx(nc.sync, xa_sb, xa, h, 512, 2048)     # q-proj ch1-3
            nc.scalar.dma_start(out=bvg_sb, in_=bvg.to_broadcast((128, 259)))
            for h in range(2):
                ldx(nc.scalar, xb_sb, xb, h, 0, 512)      # kt(0)/v(0..3)
            for q in range(3):
                for h in range(2):
                    ldx(nc.scalar, xb_sb, xb, h, 1024 + q * 1024,
                        2048 + q * 1024)
            for i in range(MT):
                nc.vector.tensor_copy(v_aug[:, i, C:258], bva_sb[:, C:258])

            # x_a in plain-f32 view for the finalize add (same bits)
            xa_f = [xa_sb[h].bitcast(F32) for h in range(2)]

            # ---- projections -----------------------------------------
            # qT[o, n] = sum_c Wq_pad[o, c] xa[c, n] ; psum [128, 512]
            def emit_q(ch):
                ns = slice(ch * 512, (ch + 1) * 512)
                ps = stp.tile([128, 512], F32, tag="st", name=f"qps_{ch}")
                for h in range(2):
                    nc.tensor.matmul(ps, wqt_sb[h], xa_sb[h][:, ns],
                                     start=(h == 0), stop=(h == 1))
                nc.vector.tensor_scalar_add(qtp[:, ns], ps, bq_sb)

            emit_q(0)  # ch1-3 interleave into the chunk-0 pair loop

            def emit_kt(mch):
                ms = slice(mch * 512, (mch + 1) * 512)
                ps = stp.tile([128, 512], F32, tag="st", name=f"ktps_{mch}")
                for h in range(2):
                    nc.tensor.matmul(ps, wkt_sb[h], xb_sb[h][:, ms],
                                     start=(h == 0), stop=(h == 1))
                nc.vector.tensor_scalar_add(kmat[:, ms], ps, bk_sb)

            # v[m, c] = sum_cc xb[cc, m] Wv[c, cc] ; lhsT = xb tile slice.
            def emit_v(i):
                ms = slice(i * 128, (i + 1) * 128)
                ps = stp.tile([128, C], F32, tag="st", name=f"vps_{i}")
                for h in range(2):
                    nc.tensor.matmul(ps, xb_sb[h][:, ms], wvt_sb[h],
                                     start=(h == 0), stop=(h == 1))
                nc.vector.tensor_add(v_aug[:, i, 0:C], ps, bva_sb[:, 0:C])

            # ---- attention main loop ---------------------------------
            def emit_st(ch, p):
                # logits for m-pair p of n-chunk ch -> 2x [128, 512] psum
                ns = slice(ch * 512, (ch + 1) * 512)
                mA, mB = 2 * p, 2 * p + 1
                ex = work.tile([128, 1024], F32R, tag="exp",
                               name=f"ex_{ch}_{p}")
                for i, m in enumerate((mA, mB)):
                    st = stp.tile([128, 512], F32, tag="st",
                                  name=f"st_{ch}_{p}_{i}")
                    nc.tensor.matmul(st, kmat[:, m * 128:(m + 1) * 128],
                                     qtp[:, ns], start=True, stop=True)
                    nc.scalar.activation(out=ex[:, i * 512:(i + 1) * 512],
                                         in_=st, func=EXP)
                return ex

            NP = MT // 2
            for ch in range(NCH):
                ops = [opp.tile([128, 258], F32, tag=f"out{j}", name=f"ops{j}")
                       for j in range(4)]
                if ch == 0:
                    emit_kt(0)
                    emit_kt(1)
                    for i in range(4):
                        emit_v(i)
                    ex_next = emit_st(0, 0)
                for p in range(NP):
                    mA, mB = 2 * p, 2 * p + 1
                    if ch == 0:
                        if 2 * p + 5 < MT:
                            emit_v(2 * p + 4)
                            emit_v(2 * p + 5)
                        nxt = (p + 1) // 2 + 1
                        if p % 2 == 1 and nxt < N // 512:
                            emit_kt(nxt)
                        if p in (3, 5, 7):
                            emit_q((p - 1) // 2)
                    ex = ex_next
                    # issue next pair's ST/exp before this pair's out-MMs so
                    # ACT(exp) overlaps PE(out) instead of serializing
                    if p + 1 < NP:
                        ex_next = emit_st(ch, p + 1)
                    elif ch + 1 < NCH:
                        ex_next = emit_st(ch + 1, 0)
                    for j in range(4):
                        js = slice(j * 128, (j + 1) * 128)
                        nc.tensor.matmul(ops[j], ex[:, js], v_aug[:, mA, :],
                                         start=(p == 0), stop=False,
                                         skip_group_check=True)
                        js2 = slice(512 + j * 128, 512 + (j + 1) * 128)
                        nc.tensor.matmul(ops[j], ex[:, js2], v_aug[:, mB, :],
                                         start=False, stop=(p == NP - 1),
                                         skip_group_check=True)

                # ---- finalize this n-chunk ---------------------------
                otiles = [outp.tile([128, 512], F32, tag=f"ot{h}", name=f"otile{h}")
                          for h in range(2)]
                for j in range(4):
                    rsum = small.tile([128, 1], F32, tag="rsum")
                    nc.vector.reciprocal(rsum, ops[j][:, 256:257])
                    sc2 = small.tile([128, 1], F32, tag="sc2")
                    nc.vector.tensor_mul(sc2, rsum, gam_sb)
                    scaled = work.tile([128, C], F32, tag="scaled")
                    nc.vector.tensor_scalar_mul(scaled, ops[j][:, 0:C], sc2)
                    for h in range(2):
                        tp = opp.tile([128, 128], F32, tag=f"out{j}",
                                      name=f"tp_{ch}_{j}_{h}")
                        nc.tensor.transpose(
                            tp, scaled[:, h * 128:(h + 1) * 128], ident)
                        nc.vector.tensor_add(
                            otiles[h][:, j * 128:(j + 1) * 128], tp,
                            xa_f[h][:, ch * 512 + j * 128:
                                    ch * 512 + (j + 1) * 128])
                for h in range(2):
                    nc.sync.dma_start(
                        out=out[h * 128:(h + 1) * 128,
                                ch * 512:(ch + 1) * 512],
                        in_=otiles[h])
    nc.compile()
    return nc


_NC_CACHE = None


def _get_nc():
    global _NC_CACHE
    if _NC_CACHE is None:
        _NC_CACHE = _build()
    return _NC_CACHE


def kernel(x_a, x_b, Wq, bq, Wk, bk, Wv, bv, gamma):
    x_a = np.ascontiguousarray(np.asarray(x_a, dtype=np.float32))
    x_b = np.ascontiguousarray(np.asarray(x_b, dtype=np.float32))
    Wq = np.asarray(Wq, dtype=np.float32)
    Wk = np.asarray(Wk, dtype=np.float32)
    Wv = np.asarray(Wv, dtype=np.float32)
    bqv = np.asarray(bq, dtype=np.float32).reshape(CK)
    bkv = np.asarray(bk, dtype=np.float32).reshape(CK)
    bvv = np.asarray(bv, dtype=np.float32).reshape(C)
    gv = float(np.asarray(gamma, dtype=np.float32).reshape(1)[0])

    xaf = x_a.reshape(B, C, N)
    xbf = x_b.reshape(B, C, N)

    wpk = np.zeros((C, WP), np.float32)
    wpk[:, 0:CK] = Wq.T
    wpk[:, 128:128 + CK] = Wk.T
    wpk[:, 256:512] = Wv.T
    wpk[0:CK, 512] = bqv
    wpk[0:CK, 513] = bkv
    bvg = np.concatenate([bvv, np.array([1.0, 0.0, gv], np.float32)]).reshape(1, 259)

    in_maps = []
    for c in range(N_CORES):
        b, half = c // 2, c % 2
        in_maps.append({
            "xa": np.ascontiguousarray(xaf[b, :, half * NH:(half + 1) * NH]),
            "xb": np.ascontiguousarray(xbf[b]),
            "wpack": wpk, "bvg": bvg,
        })

    nc = _get_nc()
    res = bass_utils.run_bass_kernel_spmd(nc, in_maps,
                                          core_ids=list(range(N_CORES)))
    out = np.empty((B, C, N), np.float32)
    for c in range(N_CORES):
        b, half = c // 2, c % 2
        out[b, :, half * NH:(half + 1) * NH] = res.results[c]["out"]
    return out.reshape(B, C, H, W)


# revision 26
# speedup vs baseline: 1.0190x; 1.0190x over previous
"""Cross-attention kernel for Trainium2 (8 NeuronCores, SPMD).

Problem: out = x_a + gamma * attn_out where
  q = Wq @ xa + bq   [B, N, CK]     (1x1 conv == per-pixel linear)
  k = Wk @ xb + bk   [B, CK, N]
  v = Wv @ xb + bv   [B, N, C]
  attn_out = softmax(q @ k, axis=-1) @ v   (transposed back to [B, C, H, W])
with B=4, C=256, CK=32, N=64*64=4096.

Sharding: 8 cores = (batch b, n-half) pairs. Each core computes q for its
2048 rows, full k/v for its batch (replicated work within a batch pair),
and its 2048xN attention rows locally. No cross-core communication.

On-core dataflow (all matmuls in float32r, 1 PE cycle/row):
  All weights/biases arrive as ONE host-packed, host-zero-padded tensor
  (wpack [C, 514] = [WqT|0pad | WkT|0pad | WvT | bq | bk]) so SBUF setup
  needs two DMAs and zero memsets; per-DMA queue overhead (~0.6us each,
  serialized) is what gates the first matmul.
  qT [128, n] = WqT_pad.T @ xa    kmat [128, m] = WkT_pad.T @ xb
  (padding the weights' free dim to K=128 keeps the fast K=128 PE path
  and writes all 128 partitions of qtp/kmat directly)
  ST tile [m=128, n=512] pairs = kmat_block.T @ qT_pad
  expST = Exp(ST) on ScalarE (softmax without max-subtraction: logits are
  bounded ~|47| here, exp stays finite in fp32)
  out_aug [n, 258] += expST.T @ v_aug  where v_aug = [v | ones | pad]
  => column 256 accumulates the softmax denominator for free.
  finalize: scale rows by gamma/rowsum (gpsimd), PE-transpose to [c, n],
  add x_a (f32 view of the xa tile already in SBUF), DMA out in [C, N]
  layout.
DMA: xa on the SP HWDGE queue, xb on the Activation HWDGE queue, in
[128, 1024] chunks in consumption order.
"""
import numpy as np

import concourse.bass as bass
import concourse.mybir as mybir
import concourse.tile as tile
from concourse import bacc, bass_utils
from concourse.masks import make_identity

F32 = mybir.dt.float32
F32R = mybir.dt.float32r
EXP = mybir.ActivationFunctionType.Exp
CPY = mybir.ActivationFunctionType.Identity
BF16 = mybir.dt.bfloat16

B, C, H, W = 4, 256, 64, 64
N = H * W            # 4096 keys per batch
CK = 32              # q/k projection dim
NH = N // 2          # 2048 query rows per core
N_CORES = 8
NCH = NH // 512      # 4 n-chunks of 512 per core
MT = N // 128        # 32 m-tiles of 128
WP = 514             # packed weights: 128 wq | 128 wk | 256 wv | bq | bk


def _build():
    nc = bacc.Bacc("TRN2", target_bir_lowering=False, debug=False,
                   enable_asserts=False)
    xa = nc.dram_tensor("xa", [C, NH], F32R, kind="ExternalInput").ap()
    xb = nc.dram_tensor("xb", [C, N], F32R, kind="ExternalInput").ap()
    wpack = nc.dram_tensor("wpack", [C, WP], F32R, kind="ExternalInput").ap()
    bvg = nc.dram_tensor("bvg", [1, 259], F32, kind="ExternalInput").ap()
    out = nc.dram_tensor("out", [C, NH], F32, kind="ExternalOutput").ap()

    with tile.TileContext(nc) as tc:
        with tc.tile_pool(name="const", bufs=1) as const, \
             tc.tile_pool(name="work", bufs=3) as work, \
             tc.tile_pool(name="outp", bufs=2) as outp, \
             tc.tile_pool(name="small", bufs=4) as small, \
             tc.tile_pool(name="stp", bufs=4, space="PSUM") as stp, \
             tc.tile_pool(name="opp", bufs=1, space="PSUM") as opp:

            # ---- constants / persistent tiles -------------------------
            xa_sb = [const.tile([128, NH], F32R, tag=f"xa{h}", name=f"xa_sb{h}") for h in range(2)]
            xb_sb = [const.tile([128, N], F32R, tag=f"xb{h}", name=f"xb_sb{h}") for h in range(2)]
            wp_sb = [const.tile([128, WP], F32R, tag=f"wp{h}", name=f"wp_sb{h}") for h in range(2)]
            CS = [slice(0, 128), slice(128, 256)]
            kmat = const.tile([128, N], F32R, tag="kmat")
            qtp = const.tile([128, NH], F32R, tag="qtp")
            v_aug = const.tile([128, MT, 258], BF16, tag="vaug")
            bvg_sb = const.tile([128, 259], F32, tag="bvg")
            ident = const.tile([128, 128], BF16, tag="ident")

            wqt_sb = [wp_sb[h][:, 0:128] for h in range(2)]
            wkt_sb = [wp_sb[h][:, 128:256] for h in range(2)]
            wvt_sb = [wp_sb[h][:, 256:512] for h in range(2)]
            bq_sb = wp_sb[0].bitcast(F32)[:, 512:513]
            bk_sb = wp_sb[0].bitcast(F32)[:, 513:514]
            bva_sb = bvg_sb[:, 0:258]
            gam_sb = bvg_sb[:, 258:259]

            make_identity(nc, ident)
            # keep the PE p-state ramp warm while the first DMAs land
            warm = stp.tile([128, 128], F32, tag="st", name="warm")
            for _ in range(16):
                nc.tensor.matmul(warm, ident, ident, start=True, stop=True,
                                 skip_group_check=True)

            # Two HWDGE queues, DMAs in consumption order with small leading
            # chunks: first ST pair needs wp + xa[:512] + xb[:512] + bvg only.
            def ldx(eng, dst, src, h, lo, hi):
                eng.dma_start(out=dst[h][:, lo:hi], in_=src[CS[h], lo:hi])
            for h in range(2):
                nc.sync.dma_start(out=wp_sb[h], in_=wpack[CS[h], :])
            for h in range(2):
                ldx(nc.sync, xa_sb, xa, h, 0, 512)        # q-proj ch0
            for h in range(2):
                ldx(nc.sync, xb_sb, xb, h, 512, 1024)     # kt(1)/v(4..7)
            for h in range(2):
                ldx(nc.sync, xa_sb, xa, h, 512, 2048)     # q-proj ch1-3
            nc.scalar.dma_start(out=bvg_sb, in_=bvg.to_broadcast((128, 259)))
            for h in range(2):
                ldx(nc.scalar, xb_sb, xb, h, 0, 512)      # kt(0)/v(0..3)
            for q in range(3):
                for h in range(2):
                    ldx(nc.scalar, xb_sb, xb, h, 1024 + q * 1024,
                        2048 + q * 1024)
            nc.gpsimd.memset(v_aug[:, :, 256:257], 1.0)
            nc.gpsimd.memset(v_aug[:, :, 257:258], 0.0)

            # x_a in plain-f32 view for the finalize add (same bits)
            xa_f = [xa_sb[h].bitcast(F32) for h in range(2)]

            # ---- projections -----------------------------------------
            # qT[o, n] = sum_c Wq_pad[o, c] xa[c, n] ; psum [128, 512]
            def emit_q(ch):
                ns = slice(ch * 512, (ch + 1) * 512)
                ps = stp.tile([128, 512], F32, tag="st", name=f"qps_{ch}")
                for h in range(2):
                    nc.tensor.matmul(ps, wqt_sb[h], xa_sb[h][:, ns],
                                     start=(h == 0), stop=(h == 1))
                nc.vector.tensor_scalar_add(qtp[:, ns], ps, bq_sb)

            emit_q(0)  # ch1-3 interleave into the chunk-0 pair loop

            def emit_kt(mch):
                ms = slice(mch * 512, (mch + 1) * 512)
                ps = stp.tile([128, 512], F32, tag="st", name=f"ktps_{mch}")
                for h in range(2):
                    nc.tensor.matmul(ps, wkt_sb[h], xb_sb[h][:, ms],
                                     start=(h == 0), stop=(h == 1))
                nc.vector.tensor_scalar_add(kmat[:, ms], ps, bk_sb)

            # v[m, c] = sum_cc xb[cc, m] Wv[c, cc] ; lhsT = xb tile slice.
            def emit_v(i):
                ms = slice(i * 128, (i + 1) * 128)
                ps = stp.tile([128, C], F32, tag="st", name=f"vps_{i}")
                for h in range(2):
                    nc.tensor.matmul(ps, xb_sb[h][:, ms], wvt_sb[h],
                                     start=(h == 0), stop=(h == 1))
                nc.vector.tensor_add(v_aug[:, i, 0:C], ps, bva_sb[:, 0:C])

            # ---- attention main loop ---------------------------------
            def emit_st(ch, p):
                # logits for m-pair p of n-chunk ch -> 2x [128, 512] psum
                ns = slice(ch * 512, (ch + 1) * 512)
                mA, mB = 2 * p, 2 * p + 1
                ex = work.tile([128, 1024], BF16, tag="exp",
                               name=f"ex_{ch}_{p}")
                for i, m in enumerate((mA, mB)):
                    st = stp.tile([128, 512], F32, tag="st",
                                  name=f"st_{ch}_{p}_{i}")
                    nc.tensor.matmul(st, kmat[:, m * 128:(m + 1) * 128],
                                     qtp[:, ns], start=True, stop=True)
                    nc.scalar.activation(out=ex[:, i * 512:(i + 1) * 512],
                                         in_=st, func=EXP)
                return ex

            NP = MT // 2
            for ch in range(NCH):
                ops = [opp.tile([128, 258], F32, tag=f"out{j}", name=f"ops{j}")
                       for j in range(4)]
                if ch == 0:
                    emit_kt(0)
                    emit_kt(1)
                    for i in range(4):
                        emit_v(i)
                    ex_next = emit_st(0, 0)
                for p in range(NP):
                    mA, mB = 2 * p, 2 * p + 1
                    if ch == 0:
                        if 2 * p + 5 < MT:
                            emit_v(2 * p + 4)
                            emit_v(2 * p + 5)
                        nxt = (p + 1) // 2 + 1
                        if p % 2 == 1 and nxt < N // 512:
                            emit_kt(nxt)
                        if p in (3, 5, 7):
                            emit_q((p - 1) // 2)
                    ex = ex_next
                    # issue next pair's ST/exp before this pair's out-MMs so
                    # ACT(exp) overlaps PE(out) instead of serializing
                    if p + 1 < NP:
                        ex_next = emit_st(ch, p + 1)
                    elif ch + 1 < NCH:
                        ex_next = emit_st(ch + 1, 0)
                    for j in range(4):
                        js = slice(j * 128, (j + 1) * 128)
                        nc.tensor.matmul(ops[j], ex[:, js], v_aug[:, mA, :],
                                         start=(p == 0), stop=False,
                                         skip_group_check=True)
                        js2 = slice(512 + j * 128, 512 + (j + 1) * 128)
                        nc.tensor.matmul(ops[j], ex[:, js2], v_aug[:, mB, :],
                                         start=False, stop=(p == NP - 1),
                                         skip_group_check=True)

                # ---- finalize this n-chunk ---------------------------
                otiles = [outp.tile([128, 512], F32, tag=f"ot{h}", name=f"otile{h}")
                          for h in range(2)]
                for j in range(4):
                    rsum = small.tile([128, 1], F32, tag="rsum")
                    nc.vector.reciprocal(rsum, ops[j][:, 256:257])
                    sc2 = small.tile([128, 1], F32, tag="sc2")
                    nc.vector.tensor_mul(sc2, rsum, gam_sb)
                    scaled = work.tile([128, C], BF16, tag="scaled")
                    if ch == NCH - 1 and j % 2 == 1:
                        nc.scalar.activation(out=scaled, in_=ops[j][:, 0:C],
                                             func=CPY, scale=sc2)
                    else:
                        nc.vector.tensor_scalar_mul(scaled, ops[j][:, 0:C],
                                                    sc2)
                    for h in range(2):
                        tp = opp.tile([128, 128], BF16, tag=f"out{j}",
                                      name=f"tp_{ch}_{j}_{h}")
                        nc.tensor.transpose(
                            tp, scaled[:, h * 128:(h + 1) * 128], ident)
                        nc.vector.tensor_add(
                            otiles[h][:, j * 128:(j + 1) * 128], tp,
                            xa_f[h][:, ch * 512 + j * 128:
                                    ch * 512 + (j + 1) * 128])
                for h in range(2):
                    nc.sync.dma_start(
                        out=out[h * 128:(h + 1) * 128,
                                ch * 512:(ch + 1) * 512],
                        in_=otiles[h])
    nc.compile()
    return nc


_NC_CACHE = None


def _get_nc():
    global _NC_CACHE
    if _NC_CACHE is None:
        _NC_CACHE = _build()
    return _NC_CACHE


def kernel(x_a, x_b, Wq, bq, Wk, bk, Wv, bv, gamma):
    x_a = np.ascontiguousarray(np.asarray(x_a, dtype=np.float32))
    x_b = np.ascontiguousarray(np.asarray(x_b, dtype=np.float32))
    Wq = np.asarray(Wq, dtype=np.float32)
    Wk = np.asarray(Wk, dtype=np.float32)
    Wv = np.asarray(Wv, dtype=np.float32)
    bqv = np.asarray(bq, dtype=np.float32).reshape(CK)
    bkv = np.asarray(bk, dtype=np.float32).reshape(CK)
    bvv = np.asarray(bv, dtype=np.float32).reshape(C)
    gv = float(np.asarray(gamma, dtype=np.float32).reshape(1)[0])

    xaf = x_a.reshape(B, C, N)
    xbf = x_b.reshape(B, C, N)

    wpk = np.zeros((C, WP), np.float32)
    wpk[:, 0:CK] = Wq.T
    wpk[:, 128:128 + CK] = Wk.T
    wpk[:, 256:512] = Wv.T
    wpk[0:CK, 512] = bqv
    wpk[0:CK, 513] = bkv
    bvg = np.concatenate([bvv, np.array([1.0, 0.0, gv], np.float32)]).reshape(1, 259)

    in_maps = []
    for c in range(N_CORES):
        b, half = c // 2, c % 2
        in_maps.append({
            "xa": np.ascontiguousarray(xaf[b, :, half * NH:(half + 1) * NH]),
            "xb": np.ascontiguousarray(xbf[b]),
            "wpack": wpk, "bvg": bvg,
        })

    nc = _get_nc()
    res = bass_utils.run_bass_kernel_spmd(nc, in_maps,
                                          core_ids=list(range(N_CORES)))
    out = np.empty((B, C, N), np.float32)
    for c in range(N_CORES):
        b, half = c // 2, c % 2
        out[b, :, half * NH:(half + 1) * NH] = res.results[c]["out"]
    return out.reshape(B, C, H, W)
